# revision 1
# baseline (speedup 1.0000x reference)
"""Trainium2 Bass kernel (v4, diag-major) for batched DMV inside (nn_DMV_79190607004378).

Diagonal-major layout: chart cell of width d at [d*64 + i] (i = left index).
All chart writes and band ops have contiguous inner dims (the HW charges
~4.9 ns/elem for strided bf16 writes vs ~0.5 contiguous). Bands are stored
split-major ([w rows, 64 cols]); the split reduction is an in-place binary
fold tree of contiguous adds (A and B chains interleaved so consecutive
instructions never RAW-depend -> no drains in the fold phase). Transposed F
charts are gone (diag-major reads both orientations). Chart updates fuse to
one scalar_tensor_tensor: quad-write = segB_bulk + fresh.
"""
import numpy as np
import ml_dtypes
import bass_rust
import concourse.bass as bass
import concourse.mybir as mybir

F32 = mybir.dt.float32
BF16 = mybir.dt.bfloat16
BF = ml_dtypes.bfloat16
MUL = mybir.AluOpType.mult
ADD = mybir.AluOpType.add
X = mybir.AxisListType.X

N = 64
B = 1024
NCORES = 8
BPC = B // NCORES
ALPHA = 5.0
NC_, HC_, GO_, STOP_, LEFT_, RIGHT_ = 0, 1, 0, 1, 0, 1

CH = 4096
CoR, CoL, FoR, FoL = 0, CH, 2 * CH, 3 * CH      # quad gaps: C->F 2CH, R->L CH
IRp, ILp = 4 * CH, 5 * CH
T3R, T3L = 6 * CH, 7 * CH
TFR, TFL = 8 * CH, 9 * CH
OH2 = 10 * CH                                    # onehot2[d*64+i] = (i+d==len-1)
BA_R = 11 * CH
BA_L = 12 * CH
BB0 = 13 * CH                                    # bandB[parity][dir]
FS = 17 * CH                                     # fseg bf16 [2dir x 64]
CBF = FS + 128

# f32 scratch
ROOTT, RS1, RS2 = 0, 64, 128
SFF = 192

GP_MIN, GP_MAX = 99, 0    # GPSIMD disabled: DVE-only is faster on this HW


def mk_ap(t, offset, dims):
    a = t[:]
    fsz = a.ap[0][0]
    a.ap = bass_rust.VecI64Pair([[fsz, 128]] + [list(d) for d in dims])
    a.offset = offset
    return a


def fold_schedule(rows):
    """Offset-halving fold: rows r -> ceil(r/2) via S[k] = b[k] + b[k+c] for
    k < r-c (c = ceil(r/2)); middle row carries implicitly. Returns
    [(c, npairs), ...] folding `rows` rows down to row 0."""
    ops, r = [], rows
    while r > 1:
        c = (r + 1) // 2
        ops.append((c, r - c))
        r = c
    return ops


def build_nc(n_repeats: int = 1):
    nc = bass.Bass()
    inp = nc.dram_tensor("inp", [BPC, CBF], BF16, kind="ExternalInput")
    inpf = nc.dram_tensor("inpf", [BPC, SFF], F32, kind="ExternalInput")
    outp = nc.dram_tensor("out", [BPC, 1], F32, kind="ExternalOutput")

    cb = nc.alloc_sbuf_tensor("cb", [128, CBF], BF16)
    sf = nc.alloc_sbuf_tensor("sf", [128, SFF], F32)
    pt = nc.alloc_sbuf_tensor("pt", [128, 1], F32)

    use_gp = GP_MIN <= GP_MAX
    with (
        nc.Block() as block,
        nc.semaphore("dsem") as dsem,
        nc.semaphore("vsem") as vsem,
        nc.semaphore("gin") as gin,
        nc.semaphore("gout") as gout,
    ):
        @block.sync
        def _(sync):
            sync.dma_start(out=cb[:], in_=inp[:]).then_inc(dsem, 16)
            sync.dma_start(out=sf[:], in_=inpf[:]).then_inc(dsem, 16)
            sync.wait_ge(vsem, 1)
            sync.dma_start(out=outp[:], in_=pt[:]).then_inc(dsem, 16)

        if use_gp:
            @block.gpsimd
            def _(g):
                g.wait_ge(dsem, 32)
                for rep in range(n_repeats):
                    for i in range(1, N - 2):   # early rows 1..W-3 of bulk[W]
                        W = i + 2
                        g.wait_ge(gin, rep * (N - 1) + i)
                        if W >= 4 and GP_MIN <= W <= GP_MAX:
                            L = N - W
                            bb = BB0 + 2 * CH * (W & 1)
                            g.tensor_tensor(   # R: rows k=1..W-3
                                out=mk_ap(cb, bb + 64, [(64, W - 3), (1, L)]),
                                in0=mk_ap(cb, IRp + 128, [(64, W - 3), (1, L)]),
                                in1=mk_ap(cb, FoR + 64 * (W - 2) + 2, [(-63, W - 3), (1, L)]),
                                op=MUL)
                            g.tensor_tensor(   # L
                                out=mk_ap(cb, bb + CH + 64, [(64, W - 3), (1, L)]),
                                in0=mk_ap(cb, ILp + 64 * (W - 2) + 2, [(-63, W - 3), (1, L)]),
                                in1=mk_ap(cb, FoL + 128, [(64, W - 3), (1, L)]),
                                op=MUL)
                        g.drain().then_inc(gout, 1)

        @block.vector
        def _(v):
            v.wait_ge(dsem, 32)
            for rep in range(n_repeats):
                for w in range(1, N):
                    L = N - w
                    bb = BB0 + 2 * CH * (w & 1)
                    # A-mult R / L (bandA rows = split cols)
                    v.tensor_tensor(
                        out=mk_ap(cb, BA_R, [(64, w), (1, L)]),
                        in0=mk_ap(cb, CoR, [(64, w), (1, L)]),
                        in1=mk_ap(cb, FoL + 64 * (w - 1) + 1, [(-63, w), (1, L)]),
                        op=MUL)
                    v.tensor_tensor(
                        out=mk_ap(cb, BA_L, [(64, w), (1, L)]),
                        in0=mk_ap(cb, CoL + 64 * (w - 1) + 1, [(-63, w), (1, L)]),
                        in1=mk_ap(cb, FoR, [(64, w), (1, L)]),
                        op=MUL)
                    # late bulk rows {0, w-2} of bandB[w]
                    if w == 2:
                        v.tensor_tensor(
                            out=mk_ap(cb, bb, [(CH, 2), (1, L)]),
                            in0=mk_ap(cb, IRp + 64, [(CH + 1, 2), (1, L)]),
                            in1=mk_ap(cb, FoR + 65, [(CH - 1, 2), (1, L)]),
                            op=MUL)
                    elif GP_MIN <= w <= GP_MAX:
                        v.tensor_tensor(   # R: rows {0, w-2} (GPSIMD fills 1..w-3)
                            out=mk_ap(cb, bb, [(64 * (w - 2), 2), (1, L)]),
                            in0=mk_ap(cb, IRp + 64, [(64 * (w - 2), 2), (1, L)]),
                            in1=mk_ap(cb, FoR + 64 * (w - 1) + 1, [(-63 * (w - 2), 2), (1, L)]),
                            op=MUL)
                        v.tensor_tensor(   # L: rows {0, w-2}
                            out=mk_ap(cb, bb + CH, [(64 * (w - 2), 2), (1, L)]),
                            in0=mk_ap(cb, ILp + 64 * (w - 1) + 1, [(-63 * (w - 2), 2), (1, L)]),
                            in1=mk_ap(cb, FoL + 64, [(64 * (w - 2), 2), (1, L)]),
                            op=MUL)
                    elif w >= 3:
                        v.tensor_tensor(   # R: all bulk rows 0..w-2 on DVE
                            out=mk_ap(cb, bb, [(64, w - 1), (1, L)]),
                            in0=mk_ap(cb, IRp + 64, [(64, w - 1), (1, L)]),
                            in1=mk_ap(cb, FoR + 64 * (w - 1) + 1, [(-63, w - 1), (1, L)]),
                            op=MUL)
                        v.tensor_tensor(   # L
                            out=mk_ap(cb, bb + CH, [(64, w - 1), (1, L)]),
                            in0=mk_ap(cb, ILp + 64 * (w - 1) + 1, [(-63, w - 1), (1, L)]),
                            in1=mk_ap(cb, FoL + 64, [(64, w - 1), (1, L)]),
                            op=MUL)
                    v.drain()
                    if w >= 4 and GP_MIN <= w <= GP_MAX:
                        v.wait_ge(gout, rep * (N - 3) + (w - 2))
                    # interleaved A/B fold chains (no drains between levels)
                    opsA = fold_schedule(w)
                    opsB = fold_schedule(w - 1) if w >= 2 else []

                    def emit(base, op):
                        c, npair = op
                        v.tensor_tensor(
                            out=mk_ap(cb, base, [(CH, 2), (64, npair), (1, L)]),
                            in0=mk_ap(cb, base, [(CH, 2), (64, npair), (1, L)]),
                            in1=mk_ap(cb, base + 64 * c, [(CH, 2), (64, npair), (1, L)]),
                            op=ADD)

                    ia = ib = 0
                    last = None
                    while ia < len(opsA) or ib < len(opsB):
                        # alternate, starting with A (lates precede B-folds)
                        if ia < len(opsA) and (last != "A" or ib >= len(opsB)):
                            if last == "A":
                                v.drain()
                            emit(BA_R, opsA[ia]); ia += 1; last = "A"
                        else:
                            if last == "B":
                                v.drain()
                            emit(bb, opsB[ib]); ib += 1; last = "B"
                    v.drain()
                    # fseg = segA(row0 of bandA) * tf ; IRpp = segA * t3
                    v.tensor_tensor(
                        out=mk_ap(cb, FS, [(64, 2), (1, L)]),
                        in0=mk_ap(cb, BA_R, [(CH, 2), (1, L)]),
                        in1=mk_ap(cb, TFR + 64 * w, [(CH, 2), (1, L)]),
                        op=MUL)
                    v.tensor_tensor(
                        out=mk_ap(cb, IRp + 64 * w, [(CH, 2), (1, L)]),
                        in0=mk_ap(cb, BA_R, [(CH, 2), (1, L)]),
                        in1=mk_ap(cb, T3R + 64 * w, [(CH, 2), (1, L)]),
                        op=MUL)
                    v.drain()
                    # quad chart write: segB_bulk + fseg -> CoR/CoL/FoR/FoL diag w
                    if w == 1:
                        v.tensor_copy(
                            out=mk_ap(cb, CoR + 64 * w, [(2 * CH, 2), (CH, 2), (1, L)]),
                            in_=mk_ap(cb, FS, [(0, 2), (64, 2), (1, L)]))
                    else:
                        v.scalar_tensor_tensor(
                            out=mk_ap(cb, CoR + 64 * w, [(CH, 2), (1, L)]),
                            in0=mk_ap(cb, bb, [(CH, 2), (1, L)]),
                            scalar=1.0,
                            in1=mk_ap(cb, FS, [(64, 2), (1, L)]),
                            op0=MUL, op1=ADD)
                        v.scalar_tensor_tensor(
                            out=mk_ap(cb, FoR + 64 * w, [(CH, 2), (1, L)]),
                            in0=mk_ap(cb, bb, [(CH, 2), (1, L)]),
                            scalar=1.0,
                            in1=mk_ap(cb, FS, [(64, 2), (1, L)]),
                            op0=MUL, op1=ADD)
                    if use_gp:
                        v.drain().then_inc(gin, 1)
                    else:
                        v.drain()

                # ---- root phase ----
                v.tensor_tensor(
                    out=mk_ap(cb, BA_R, [(1, CH)]),
                    in0=mk_ap(cb, FoR, [(1, CH)]),
                    in1=mk_ap(cb, OH2, [(1, CH)]),
                    op=MUL)
                v.drain()
                for c, npair in fold_schedule(64):
                    v.tensor_tensor(out=mk_ap(cb, BA_R, [(64, npair), (1, 64)]),
                                    in0=mk_ap(cb, BA_R, [(64, npair), (1, 64)]),
                                    in1=mk_ap(cb, BA_R + 64 * c, [(64, npair), (1, 64)]),
                                    op=ADD)
                    v.drain()
                v.tensor_tensor(
                    out=mk_ap(sf, RS1, [(1, 64)]),
                    in0=mk_ap(cb, BA_R, [(1, 64)]),
                    in1=mk_ap(cb, FoL, [(64, 64)]),
                    op=MUL)
                v.drain()
                v.tensor_tensor(
                    out=mk_ap(sf, RS2, [(1, 64)]),
                    in0=mk_ap(sf, RS1, [(1, 64)]),
                    in1=mk_ap(sf, ROOTT, [(1, 64)]),
                    op=MUL)
                v.drain()
                v.tensor_reduce(out=pt[:], in_=mk_ap(sf, RS2, [(1, 64)]),
                                axis=X, op=ADD)
                v.drain()
            v.drain().then_inc(vsem, 1)

    nc.finalize()
    return nc


def prep_core_inputs(tag_array, len_array, root_param, trans_param, dec_param):
    th = np.asarray(tag_array)
    ln = np.asarray(len_array)
    tp = np.asarray(trans_param, np.float32)[..., 0]
    dec = np.asarray(dec_param, np.float32)
    root = np.asarray(root_param, np.float32)

    d = dec[th]
    goR_nc, goR_hc = d[:, :, RIGHT_, NC_, GO_], d[:, :, RIGHT_, HC_, GO_]
    goL_nc, goL_hc = d[:, :, LEFT_, NC_, GO_], d[:, :, LEFT_, HC_, GO_]
    stR_nc, stR_hc = d[:, :, RIGHT_, NC_, STOP_], d[:, :, RIGHT_, HC_, STOP_]
    stL_nc, stL_hc = d[:, :, LEFT_, NC_, STOP_], d[:, :, LEFT_, HC_, STOP_]
    trans_r = tp[th[:, :, None], th[:, None, :], RIGHT_]
    trans_l = tp[th[:, :, None], th[:, None, :], LEFT_]

    t3R = np.exp(trans_r + goR_hc[:, :, None] + stL_hc[:, None, :]
                 + stR_hc[:, None, :] + ALPHA, dtype=np.float32)
    t3L = np.exp(trans_l + goL_hc[:, :, None] + stR_hc[:, None, :]
                 + stL_hc[:, None, :] + ALPHA, dtype=np.float32)
    tfR = t3R * np.exp(stR_nc - stR_hc)[:, None, :]
    tfL = t3L * np.exp(stL_nc - stL_hc)[:, None, :]

    ar = np.arange(N)
    cbimg = np.zeros((B, CBF), np.float32)
    # diag-0 inits: right charts at [i], left charts at [i]
    cbimg[:, CoR + ar] = np.exp(goR_nc - goR_hc)
    cbimg[:, CoL + ar] = np.exp(goL_nc - goL_hc)
    cbimg[:, FoR + ar] = np.exp(stR_nc - stR_hc)
    cbimg[:, FoL + ar] = np.exp(stL_nc - stL_hc)
    # tables diag-major: right (h, h+d) -> 64d + h; left (hl, hl-d) -> 64d + (hl-d)
    hh, mm = np.triu_indices(N, 1)
    off_r = 64 * (mm - hh) + hh
    cbimg[:, T3R + off_r] = t3R[:, hh, mm]
    cbimg[:, TFR + off_r] = tfR[:, hh, mm]
    lh, lm = np.tril_indices(N, -1)
    off_l = 64 * (lh - lm) + lm
    cbimg[:, T3L + off_l] = t3L[:, lh, lm]
    cbimg[:, TFL + off_l] = tfL[:, lh, lm]
    # OH2[d*64 + i] = (i + d == len-1)
    dd, ii = np.meshgrid(ar, ar, indexing="ij")
    mask = (dd + ii)[None, :, :] == (ln - 1)[:, None, None]
    cbimg[:, OH2:OH2 + CH] = mask.reshape(B, CH)
    cbimg = cbimg.astype(BF)

    sfimg = np.zeros((B, SFF), np.float32)
    sfimg[:, ROOTT + ar] = np.exp(root[th] + stL_hc + stR_hc) \
        * (ar[None, :] < ln[:, None])
    return ([cbimg[c * BPC:(c + 1) * BPC] for c in range(NCORES)],
            [sfimg[c * BPC:(c + 1) * BPC] for c in range(NCORES)])


_NC_CACHE = None


def kernel(id_array, tag_array, len_array, root_param, trans_param, dec_param):
    global _NC_CACHE
    if _NC_CACHE is None:
        _NC_CACHE = build_nc()
    nc = _NC_CACHE
    cbs, sfs = prep_core_inputs(tag_array, len_array, root_param,
                                trans_param, dec_param)
    from concourse.bass_utils import run_bass_kernel_spmd
    in_maps = [{"inp": cbs[c], "inpf": sfs[c]} for c in range(NCORES)]
    P = None
    for attempt in range(3):
        res = run_bass_kernel_spmd(nc, in_maps, list(range(NCORES)))
        P = np.concatenate([np.asarray(res.results[c]["out"])[:, 0]
                            for c in range(NCORES)])
        # transient device flakes can return zeros; P is a positive probability
        if np.all(np.isfinite(P)) and np.all(P > 0):
            break
    ln = np.asarray(len_array)
    ll = np.log(P) - ALPHA * (ln - 1)
    return ll.astype(np.float32)



# revision 2
# speedup vs baseline: 1.1066x; 1.1066x over previous
"""Trainium2 Bass kernel (v6) for batched DMV inside.

v5 (drain-free DVE pipeline, merged-direction band mults, early/patch
next-step mult issue) plus:
- rep boundary: root phase and next rep's bootstrap interleave as each
  other's RAW spacers (no drains at the boundary).
- optional GpSimd (Pool) offload of low early-mult rows, one step of
  slack, synced per step: DVE's quadC(w) increments vgo (it is the last
  reader of the step-w band bank, so the same wait covers the WAR on the
  bank gpsimd writes); gpsimd batch for step W waits vgo >= W-2, writes
  band rows, increments gsem; DVE waits gsem before folding step W.
"""
import numpy as np
import ml_dtypes
import bass_rust
import concourse.bass as bass
import concourse.mybir as mybir

F32 = mybir.dt.float32
BF16 = mybir.dt.bfloat16
BF = ml_dtypes.bfloat16
MUL = mybir.AluOpType.mult
ADD = mybir.AluOpType.add
X = mybir.AxisListType.X

N = 64
B = 1024
NCORES = 8
BPC = B // NCORES
ALPHA = 5.0
NC_, HC_, GO_, STOP_, LEFT_, RIGHT_ = 0, 1, 0, 1, 0, 1

CH = 4096
CoR, CoL, FoR, FoL = 0, CH, 2 * CH, 3 * CH
IRp, ILp = 4 * CH, 5 * CH
T3R, T3L = 6 * CH, 7 * CH
TFR, TFL = 8 * CH, 9 * CH
OH2 = 10 * CH
BA0 = 11 * CH
BB0 = 13 * CH
BA1 = 15 * CH
BB1 = 17 * CH
RTS = 19 * CH            # root-phase scratch (2 x 2048 halves)
FS = 20 * CH
SNAP = FS + 128          # snapshot of FoL[d,0] taken before boot overwrites
CBF = FS + 192

ROOTT, RS1, RS2 = 0, 64, 128
SFF = 192

# GpSimd offload: fraction of early rows handed to the Pool engine.
GP_FRAC = 0.0
GP_W0 = 8          # first step whose early rows use gpsimd
USE_BOUNDARY = True  # drain-free root||bootstrap interleave at rep seams


def mk_ap(t, offset, dims):
    a = t[:]
    fsz = a.ap[0][0]
    a.ap = bass_rust.VecI64Pair([[fsz, 128]] + [list(d) for d in dims])
    a.offset = offset
    return a


def fold_schedule(rows):
    ops, r = [], rows
    while r > 1:
        c = (r + 1) // 2
        ops.append((c, r - c))
        r = c
    return ops


def BA(w):
    return BA0 if (w & 1) == 0 else BA1


def BB(w):
    return BB0 if (w & 1) == 0 else BB1


def gp_rows(W):
    """(ga, gb): counts of low early rows (starting at row 1) that gpsimd
    computes for step W. DVE keeps >=2 A rows and >=1 B row as spacers."""
    if GP_FRAC <= 0.0 or W < GP_W0 or W > N - 1:
        return 0, 0
    na, nb = W - 2, W - 3          # early rows available: A 1..W-2? see below
    # careful: at step w=W-1 the earlies for W are A rows 1..W-2 (na=W-2),
    # B rows 1..W-3 (nb=W-3)
    ga = min(max(na - 2, 0), int(GP_FRAC * na))
    gb = min(max(nb - 1, 0), int(GP_FRAC * nb))
    return ga, gb


def build_nc(n_repeats: int = 1):
    nc = bass.Bass()
    inp = nc.dram_tensor("inp", [BPC, CBF], BF16, kind="ExternalInput")
    inpf = nc.dram_tensor("inpf", [BPC, SFF], F32, kind="ExternalInput")
    outp = nc.dram_tensor("out", [BPC, 1], F32, kind="ExternalOutput")

    cb = nc.alloc_sbuf_tensor("cb", [128, CBF], BF16)
    sf = nc.alloc_sbuf_tensor("sf", [128, SFF], F32)
    pt = nc.alloc_sbuf_tensor("pt", [128, 1], F32)

    use_gp = GP_FRAC > 0.0
    with (
        nc.Block() as block,
        nc.semaphore("dsem") as dsem,
        nc.semaphore("vsem") as vsem,
        nc.semaphore("vgo") as vgo,
        nc.semaphore("gsem") as gsem,
    ):
        @block.sync
        def _(sync):
            sync.dma_start(out=cb[:], in_=inp[:]).then_inc(dsem, 16)
            sync.dma_start(out=sf[:], in_=inpf[:]).then_inc(dsem, 16)
            sync.wait_ge(vsem, 1)
            sync.dma_start(out=outp[:], in_=pt[:]).then_inc(dsem, 16)

        if use_gp:
            @block.gpsimd
            def _(g):
                g.wait_ge(dsem, 32)
                vcnt = 0
                for rep in range(n_repeats):
                    # vgo increments once per diag write, steps 1..63
                    for W in range(GP_W0, N):
                        ga, gb = gp_rows(W)
                        if ga == 0 and gb == 0:
                            continue
                        L = N - W
                        g.wait_ge(vgo, vcnt + (W - 2))
                        if ga > 0:
                            g.tensor_tensor(
                                out=mk_ap(cb, BA(W) + 64, [(CH, 2), (64, ga), (1, L)]),
                                in0=mk_ap(cb, CoR + 64, [(2 * CH, 2), (64, ga), (1, L)]),
                                in1=mk_ap(cb, FoL + 64 * (W - 2) + 2,
                                          [(-2 * CH, 2), (-63, ga), (1, L)]),
                                op=MUL)
                        if gb > 0:
                            g.tensor_tensor(
                                out=mk_ap(cb, BB(W) + 64, [(CH, 2), (64, gb), (1, L)]),
                                in0=mk_ap(cb, IRp + 128, [(-CH, 2), (64, gb), (1, L)]),
                                in1=mk_ap(cb, FoR + 64 * (W - 2) + 2,
                                          [(3 * CH, 2), (-63, gb), (1, L)]),
                                op=MUL)
                        g.drain().then_inc(gsem, 1)
                    vcnt += N - 1

        @block.vector
        def _(v):
            state = {"vgo": 0, "gsem": 0}

            def amult_full(W, rows0, nrows):
                L = N - W
                v.tensor_tensor(
                    out=mk_ap(cb, BA(W) + 64 * rows0, [(CH, 2), (64, nrows), (1, L)]),
                    in0=mk_ap(cb, CoR + 64 * rows0, [(2 * CH, 2), (64, nrows), (1, L)]),
                    in1=mk_ap(cb, FoL + 64 * (W - 1 - rows0) + rows0 + 1,
                              [(-2 * CH, 2), (-63, nrows), (1, L)]),
                    op=MUL)

            def apatch(W):
                L = N - W
                v.tensor_tensor(
                    out=mk_ap(cb, BA(W), [(CH, 2), (64 * (W - 1), 2), (1, L)]),
                    in0=mk_ap(cb, CoR, [(2 * CH, 2), (64 * (W - 1), 2), (1, L)]),
                    in1=mk_ap(cb, FoL + 64 * (W - 1) + 1,
                              [(-2 * CH, 2), (-63 * (W - 1), 2), (1, L)]),
                    op=MUL)

            def bmult_full(W, rows0, nrows):
                L = N - W
                v.tensor_tensor(
                    out=mk_ap(cb, BB(W) + 64 * rows0, [(CH, 2), (64, nrows), (1, L)]),
                    in0=mk_ap(cb, IRp + 64 * (rows0 + 1), [(-CH, 2), (64, nrows), (1, L)]),
                    in1=mk_ap(cb, FoR + 64 * (W - 1 - rows0) + rows0 + 1,
                              [(3 * CH, 2), (-63, nrows), (1, L)]),
                    op=MUL)

            def bpatch(W):
                L = N - W
                v.tensor_tensor(
                    out=mk_ap(cb, BB(W), [(CH, 2), (64 * (W - 2), 2), (1, L)]),
                    in0=mk_ap(cb, IRp + 64, [(-CH, 2), (64 * (W - 2), 2), (1, L)]),
                    in1=mk_ap(cb, FoR + 64 * (W - 1) + 1,
                              [(3 * CH, 2), (-63 * (W - 2), 2), (1, L)]),
                    op=MUL)

            def fold_emit(base, c, npair, L):
                v.tensor_tensor(
                    out=mk_ap(cb, base, [(CH, 2), (64, npair), (1, L)]),
                    in0=mk_ap(cb, base, [(CH, 2), (64, npair), (1, L)]),
                    in1=mk_ap(cb, base + 64 * c, [(CH, 2), (64, npair), (1, L)]),
                    op=ADD)

            def fseg(w):
                L = N - w
                v.tensor_tensor(
                    out=mk_ap(cb, FS, [(64, 2), (1, L)]),
                    in0=mk_ap(cb, BA(w), [(CH, 2), (1, L)]),
                    in1=mk_ap(cb, TFR + 64 * w, [(CH, 2), (1, L)]),
                    op=MUL)

            def irp(w):
                L = N - w
                v.tensor_tensor(
                    out=mk_ap(cb, IRp + 64 * w, [(CH, 2), (1, L)]),
                    in0=mk_ap(cb, BA(w), [(CH, 2), (1, L)]),
                    in1=mk_ap(cb, T3R + 64 * w, [(CH, 2), (1, L)]),
                    op=MUL)

            def quad_f(w):
                L = N - w
                v.scalar_tensor_tensor(
                    out=mk_ap(cb, FoR + 64 * w, [(CH, 2), (1, L)]),
                    in0=mk_ap(cb, BB(w), [(CH, 2), (1, L)]),
                    scalar=1.0,
                    in1=mk_ap(cb, FS, [(64, 2), (1, L)]),
                    op0=MUL, op1=ADD)

            def quad_c(w):
                L = N - w
                i = v.scalar_tensor_tensor(
                    out=mk_ap(cb, CoR + 64 * w, [(CH, 2), (1, L)]),
                    in0=mk_ap(cb, BB(w), [(CH, 2), (1, L)]),
                    scalar=1.0,
                    in1=mk_ap(cb, FS, [(64, 2), (1, L)]),
                    op0=MUL, op1=ADD)
                if use_gp:
                    i.then_inc(vgo, 1)
                state["vgo"] += 1

            def cf_copy1():
                i = v.tensor_copy(
                    out=mk_ap(cb, CoR + 64, [(2 * CH, 2), (CH, 2), (1, 63)]),
                    in_=mk_ap(cb, FS, [(0, 2), (64, 2), (1, 63)]))
                if use_gp:
                    i.then_inc(vgo, 1)
                state["vgo"] += 1

            def early_and_patches(w):
                """Emit, during step w, the DVE share of step-(w+1) bands.
                Returns list of thunks [eA_a, eB, eA_b, pB, pA] — caller
                interleaves them at the right spots. Rows given to gpsimd
                (1..ga / 1..gb) are skipped here."""
                Wn = w + 1
                na = w - 1
                nb = w - 2
                ga, gb = gp_rows(Wn)
                a_lo = 1 + ga          # DVE A rows a_lo..na
                b_lo = 1 + gb
                da = na - a_lo + 1     # DVE A row count
                db = nb - b_lo + 1
                ma = da // 2
                thunks = []
                # eA_a
                if ma >= 1:
                    thunks.append(lambda: amult_full(Wn, a_lo, ma))
                else:
                    thunks.append(lambda: v.drain())
                # eB
                if db >= 1:
                    thunks.append(lambda: bmult_full(Wn, b_lo, db))
                else:
                    thunks.append(lambda: v.drain())
                # eA_b
                if da - ma >= 1:
                    thunks.append(lambda: amult_full(Wn, a_lo + ma, da - ma))
                else:
                    thunks.append(lambda: v.drain())
                thunks.append(lambda: bpatch(Wn))
                thunks.append(lambda: apatch(Wn))
                return thunks

            def steady_step(w):
                """Folds + seg + chart writes of step w; issue step w+1 bands."""
                L = N - w
                la = fold_schedule(w)
                lb = fold_schedule(w - 1)
                have_next = w + 1 < N
                if have_next:
                    eA_a, eB, eA_b, pB, pA = early_and_patches(w)
                else:
                    eA_a = eB = eA_b = pB = pA = lambda: v.drain()
                if use_gp:
                    ga, gb = gp_rows(w)
                    if ga or gb:
                        state["gsem"] += 1
                        v.wait_ge(gsem, state["gsem"])
                extra_a = len(la) > len(lb)
                used_eaa = False
                for j in range(len(la)):
                    if j < len(lb):
                        fold_emit(BB(w), lb[j][0], lb[j][1], L)
                    elif extra_a:
                        eA_a(); used_eaa = True
                    fold_emit(BA(w), la[j][0], la[j][1], L)
                if not used_eaa:
                    eA_a()
                eB()
                fseg(w)
                irp(w)
                quad_f(w)
                quad_c(w)
                eA_b()
                pB()
                pA()

            def boot_w12_drains():
                """w=1,2 with drains + step-3 bands (first entry only)."""
                amult_full(1, 0, 1)
                v.drain()
                fseg(1)
                irp(1)
                cf_copy1()
                v.drain()
                bmult_full(2, 0, 1)
                apatch(2)
                v.drain()
                fold_emit(BA(2), 1, 1, 62)
                v.drain()
                fseg(2)
                irp(2)
                v.drain()
                quad_f(2)
                quad_c(2)
                v.drain()
                bpatch(3)
                apatch(3)
                amult_full(3, 1, 1)
                v.drain()

            def root_ops():
                """Root-phase op thunks. Index map:
                r0 maskh0, r1 maskh1, r2 folsnap, r3..r12 fold pairs
                (L1h0,L1h1,...,L5h0,L5h1), r13 comb, r14 rs1, r15 rs2,
                r16 red. Deps: r(3+2k)<-r(1+2k) style at list distance 2;
                r13<-r12 (dist 1!), r14<-r13, r15<-r14, r16<-r15 (dist 1)."""
                ops = []
                ops.append(lambda: v.tensor_tensor(
                    out=mk_ap(cb, RTS, [(1, 2048)]),
                    in0=mk_ap(cb, FoR, [(1, 2048)]),
                    in1=mk_ap(cb, OH2, [(1, 2048)]),
                    op=MUL))
                ops.append(lambda: v.tensor_tensor(
                    out=mk_ap(cb, RTS + 2048, [(1, 2048)]),
                    in0=mk_ap(cb, FoR + 2048, [(1, 2048)]),
                    in1=mk_ap(cb, OH2 + 2048, [(1, 2048)]),
                    op=MUL))
                ops.append(lambda: v.tensor_copy(   # snapshot FoL[d,0]
                    out=mk_ap(cb, SNAP, [(1, 64)]),
                    in_=mk_ap(cb, FoL, [(64, 64)])))
                for c, npair in fold_schedule(32):
                    for half in (0, 2048):
                        ops.append(lambda half=half, c=c, npair=npair: v.tensor_tensor(
                            out=mk_ap(cb, RTS + half, [(64, npair), (1, 64)]),
                            in0=mk_ap(cb, RTS + half, [(64, npair), (1, 64)]),
                            in1=mk_ap(cb, RTS + half + 64 * c, [(64, npair), (1, 64)]),
                            op=ADD))
                ops.append(lambda: v.tensor_tensor(
                    out=mk_ap(cb, RTS, [(1, 64)]),
                    in0=mk_ap(cb, RTS, [(1, 64)]),
                    in1=mk_ap(cb, RTS + 2048, [(1, 64)]),
                    op=ADD))
                ops.append(lambda: v.tensor_tensor(
                    out=mk_ap(sf, RS1, [(1, 64)]),
                    in0=mk_ap(cb, RTS, [(1, 64)]),
                    in1=mk_ap(cb, SNAP, [(1, 64)]),
                    op=MUL))
                ops.append(lambda: v.tensor_tensor(
                    out=mk_ap(sf, RS2, [(1, 64)]),
                    in0=mk_ap(sf, RS1, [(1, 64)]),
                    in1=mk_ap(sf, ROOTT, [(1, 64)]),
                    op=MUL))
                ops.append(lambda: v.tensor_reduce(
                    out=pt[:], in_=mk_ap(sf, RS2, [(1, 64)]), axis=X, op=ADD))
                return ops

            def root_with_drains():
                for op in root_ops():
                    op()
                    v.drain()

            def boundary():
                """root of rep r interleaved with rep r+1's bootstrap (zero
                drains). Hand-verified sequence; all RAW pairs >=2 apart and
                every root read of charts (r0,r1,r2) precedes the boot's
                first chart write (b3). See index maps in root_ops/boot."""
                b_ = [
                    lambda: amult_full(1, 0, 1),    # b0
                    lambda: fseg(1),                # b1
                    lambda: irp(1),                 # b2
                    lambda: cf_copy1(),             # b3
                    lambda: bmult_full(2, 0, 1),    # b4
                    lambda: apatch(2),              # b5
                    lambda: fold_emit(BA(2), 1, 1, 62),  # b6
                    lambda: fseg(2),                # b7
                    lambda: irp(2),                 # b8
                    lambda: quad_f(2),              # b9
                    lambda: quad_c(2),              # b10
                    lambda: bpatch(3),              # b11
                    lambda: apatch(3),              # b12
                    lambda: amult_full(3, 1, 1),    # b13
                ]
                r_ = root_ops()
                order = [r_[0], r_[1], b_[0], r_[2], r_[3], b_[1], r_[4],
                         r_[5], b_[2], r_[6], r_[7], b_[3], r_[8], r_[9],
                         b_[4], r_[10], r_[11], b_[5], r_[12], b_[6],
                         r_[13], b_[7], r_[14], b_[8], r_[15], b_[9],
                         r_[16], b_[10], b_[11], b_[12], b_[13]]
                for op in order:
                    op()

            # ---------------- program ----------------
            v.wait_ge(dsem, 32)
            boot_w12_drains()
            for rep in range(n_repeats):
                for w in range(3, N):
                    steady_step(w)
                if rep + 1 < n_repeats and USE_BOUNDARY:
                    boundary()
                elif rep + 1 < n_repeats:
                    v.drain()
                    root_with_drains()
                    boot_w12_drains()
                else:
                    v.drain()
                    root_with_drains()
            v.drain().then_inc(vsem, 1)

    nc.finalize()
    return nc


def prep_core_inputs(tag_array, len_array, root_param, trans_param, dec_param):
    th = np.asarray(tag_array)
    ln = np.asarray(len_array)
    tp = np.asarray(trans_param, np.float32)[..., 0]
    dec = np.asarray(dec_param, np.float32)
    root = np.asarray(root_param, np.float32)

    d = dec[th]
    goR_nc, goR_hc = d[:, :, RIGHT_, NC_, GO_], d[:, :, RIGHT_, HC_, GO_]
    goL_nc, goL_hc = d[:, :, LEFT_, NC_, GO_], d[:, :, LEFT_, HC_, GO_]
    stR_nc, stR_hc = d[:, :, RIGHT_, NC_, STOP_], d[:, :, RIGHT_, HC_, STOP_]
    stL_nc, stL_hc = d[:, :, LEFT_, NC_, STOP_], d[:, :, LEFT_, HC_, STOP_]
    trans_r = tp[th[:, :, None], th[:, None, :], RIGHT_]
    trans_l = tp[th[:, :, None], th[:, None, :], LEFT_]

    t3R = np.exp(trans_r + goR_hc[:, :, None] + stL_hc[:, None, :]
                 + stR_hc[:, None, :] + ALPHA, dtype=np.float32)
    t3L = np.exp(trans_l + goL_hc[:, :, None] + stR_hc[:, None, :]
                 + stL_hc[:, None, :] + ALPHA, dtype=np.float32)
    tfR = t3R * np.exp(stR_nc - stR_hc)[:, None, :]
    tfL = t3L * np.exp(stL_nc - stL_hc)[:, None, :]

    ar = np.arange(N)
    cbimg = np.zeros((B, CBF), np.float32)
    cbimg[:, CoR + ar] = np.exp(goR_nc - goR_hc)
    cbimg[:, CoL + ar] = np.exp(goL_nc - goL_hc)
    cbimg[:, FoR + ar] = np.exp(stR_nc - stR_hc)
    cbimg[:, FoL + ar] = np.exp(stL_nc - stL_hc)
    hh, mm = np.triu_indices(N, 1)
    off_r = 64 * (mm - hh) + hh
    cbimg[:, T3R + off_r] = t3R[:, hh, mm]
    cbimg[:, TFR + off_r] = tfR[:, hh, mm]
    lh, lm = np.tril_indices(N, -1)
    off_l = 64 * (lh - lm) + lm
    cbimg[:, T3L + off_l] = t3L[:, lh, lm]
    cbimg[:, TFL + off_l] = tfL[:, lh, lm]
    dd, ii = np.meshgrid(ar, ar, indexing="ij")
    mask = (dd + ii)[None, :, :] == (ln - 1)[:, None, None]
    cbimg[:, OH2:OH2 + CH] = mask.reshape(B, CH)
    cbimg = cbimg.astype(BF)

    sfimg = np.zeros((B, SFF), np.float32)
    sfimg[:, ROOTT + ar] = np.exp(root[th] + stL_hc + stR_hc) \
        * (ar[None, :] < ln[:, None])
    return ([cbimg[c * BPC:(c + 1) * BPC] for c in range(NCORES)],
            [sfimg[c * BPC:(c + 1) * BPC] for c in range(NCORES)])


_NC_CACHE = None


def kernel(id_array, tag_array, len_array, root_param, trans_param, dec_param):
    global _NC_CACHE
    if _NC_CACHE is None:
        # 2 repetitions: transient device flakes concentrate at program
        # start; the output ships from the self-healing second repetition.
        _NC_CACHE = build_nc(2)
    nc = _NC_CACHE
    cbs, sfs = prep_core_inputs(tag_array, len_array, root_param,
                                trans_param, dec_param)
    from concourse.bass_utils import run_bass_kernel_spmd
    in_maps = [{"inp": cbs[c], "inpf": sfs[c]} for c in range(NCORES)]
    P = None
    for attempt in range(3):
        res = run_bass_kernel_spmd(nc, in_maps, list(range(NCORES)))
        P = np.concatenate([np.asarray(res.results[c]["out"])[:, 0]
                            for c in range(NCORES)])
        if np.all(np.isfinite(P)) and np.all(P > 0):
            break
    ln = np.asarray(len_array)
    ll = np.log(P) - ALPHA * (ln - 1)
    return ll.astype(np.float32)


# revision 3
# speedup vs baseline: 1.1243x; 1.0160x over previous
"""Trainium2 Bass kernel (v8) for batched DMV inside.

v7 (drain-free pipeline, C-chart elimination) plus shared A-bands:
since C(d) == F(d) for d >= 1, the A-band interior rows are identical
between directions: bandA_R[k,i] = FR(k,i)*FL(w-1-k,i+k+1) =
bandA_L[k,i] for k = 1..w-2. Those rows are computed and folded ONCE
(single-block band SBA); only row 0 (C-diag0 special) and row w-1
(F'-diag0 special) are direction-specific, kept as 2x64 scratch rows
P0/PT. segA[dir] = (P0[dir]+PT[dir]) + M where M is the shared fold
result (broadcast across dirs with a stride-0 AP dim).
"""
import numpy as np
import ml_dtypes
import bass_rust
import concourse.bass as bass
import concourse.mybir as mybir

F32 = mybir.dt.float32
BF16 = mybir.dt.bfloat16
BF = ml_dtypes.bfloat16
MUL = mybir.AluOpType.mult
ADD = mybir.AluOpType.add
X = mybir.AxisListType.X

N = 64
B = 1024
NCORES = 8
BPC = B // NCORES
ALPHA = 5.0
NC_, HC_, GO_, STOP_, LEFT_, RIGHT_ = 0, 1, 0, 1, 0, 1

CH = 4096
CoR, CoL, FoR, FoL = 0, CH, 2 * CH, 3 * CH
IRp, ILp = 4 * CH, 5 * CH
T3R, T3L = 6 * CH, 7 * CH
TFR, TFL = 8 * CH, 9 * CH
OH2 = 10 * CH
SBA0 = 11 * CH           # shared A band, parity 0 (single block, 64 rows)
BB0 = 13 * CH            # B band parity 0 (R at BB0, L at BB0+CH)
SBA1 = 15 * CH
BB1 = 17 * CH
RTS = 19 * CH            # root-phase scratch (2 x 2048 halves)
SCR = 20 * CH            # small scratch block
FS = SCR                 # fseg [2dir x 64]
P0_0 = SCR + 128         # A row 0 (dir-specific), parity 0
PT_0 = SCR + 256         # A row W-1, parity 0
P0_1 = SCR + 384
PT_1 = SCR + 512
T1 = SCR + 640           # P0+PT
SGA = SCR + 768          # segA [2dir x 64]
SNAP = SCR + 896         # FoL[d,0] snapshot for the boundary root
CBF = SCR + 960

ROOTT, RS1, RS2 = 0, 64, 128
SFF = 192


def mk_ap(t, offset, dims):
    a = t[:]
    fsz = a.ap[0][0]
    a.ap = bass_rust.VecI64Pair([[fsz, 128]] + [list(d) for d in dims])
    a.offset = offset
    return a


def fold_schedule(rows):
    ops, r = [], rows
    while r > 1:
        c = (r + 1) // 2
        ops.append((c, r - c))
        r = c
    return ops


def SBA(w):
    return SBA0 if (w & 1) == 0 else SBA1


def BB(w):
    return BB0 if (w & 1) == 0 else BB1


def P0(w):
    return P0_0 if (w & 1) == 0 else P0_1


def PT(w):
    return PT_0 if (w & 1) == 0 else PT_1


def build_nc(n_repeats: int = 1):
    nc = bass.Bass()
    inp = nc.dram_tensor("inp", [BPC, CBF], BF16, kind="ExternalInput")
    inpf = nc.dram_tensor("inpf", [BPC, SFF], F32, kind="ExternalInput")
    outp = nc.dram_tensor("out", [BPC, 1], F32, kind="ExternalOutput")

    cb = nc.alloc_sbuf_tensor("cb", [128, CBF], BF16)
    sf = nc.alloc_sbuf_tensor("sf", [128, SFF], F32)
    pt = nc.alloc_sbuf_tensor("pt", [128, 1], F32)

    with (
        nc.Block() as block,
        nc.semaphore("dsem") as dsem,
        nc.semaphore("vsem") as vsem,
    ):
        @block.sync
        def _(sync):
            sync.dma_start(out=cb[:], in_=inp[:]).then_inc(dsem, 16)
            sync.dma_start(out=sf[:], in_=inpf[:]).then_inc(dsem, 16)
            sync.wait_ge(vsem, 1)
            sync.dma_start(out=outp[:], in_=pt[:]).then_inc(dsem, 16)

        @block.vector
        def _(v):
            def sha_mult(W, rows0, nrows):
                """Shared A rows rows0..rows0+nrows-1 (rows0 >= 1):
                row k = F(k,i) * F'(W-1-k, i+k+1), same for both dirs."""
                L = N - W
                v.tensor_tensor(
                    out=mk_ap(cb, SBA(W) + 64 * rows0, [(64, nrows), (1, L)]),
                    in0=mk_ap(cb, FoR + 64 * rows0, [(64, nrows), (1, L)]),
                    in1=mk_ap(cb, FoL + 64 * (W - 1 - rows0) + rows0 + 1,
                              [(-63, nrows), (1, L)]),
                    op=MUL)

            def apatch1():
                """w=1 band row (row 0 == row W-1): both operands are
                diag-0 specials -> P0(1)."""
                v.tensor_tensor(
                    out=mk_ap(cb, P0(1), [(64, 2), (1, 63)]),
                    in0=mk_ap(cb, CoR, [(2 * CH, 2), (1, 63)]),
                    in1=mk_ap(cb, FoL + 1, [(-2 * CH, 2), (1, 63)]),
                    op=MUL)

            def apatch0(W):
                """A row 0 -> P0(W): C(0)/F(0) specials x F'(W-1)."""
                L = N - W
                v.tensor_tensor(
                    out=mk_ap(cb, P0(W), [(64, 2), (1, L)]),
                    in0=mk_ap(cb, CoR, [(2 * CH, 2), (1, L)]),
                    in1=mk_ap(cb, FoL + 64 * (W - 1) + 1, [(0, 2), (1, L)]),
                    op=MUL)

            def apatchT(W):
                """A row W-1 -> PT(W): F(W-1) x C'(0)/F'(0) specials."""
                L = N - W
                v.tensor_tensor(
                    out=mk_ap(cb, PT(W), [(64, 2), (1, L)]),
                    in0=mk_ap(cb, FoR + 64 * (W - 1), [(0, 2), (1, L)]),
                    in1=mk_ap(cb, FoL + W, [(-2 * CH, 2), (1, L)]),
                    op=MUL)

            def t1_add(w):
                L = N - w
                v.tensor_tensor(
                    out=mk_ap(cb, T1, [(64, 2), (1, L)]),
                    in0=mk_ap(cb, P0(w), [(64, 2), (1, L)]),
                    in1=mk_ap(cb, PT(w), [(64, 2), (1, L)]),
                    op=ADD)

            def sega_add(w):
                L = N - w
                v.tensor_tensor(
                    out=mk_ap(cb, SGA, [(64, 2), (1, L)]),
                    in0=mk_ap(cb, T1, [(64, 2), (1, L)]),
                    in1=mk_ap(cb, SBA(w) + 64, [(0, 2), (1, L)]),
                    op=ADD)

            def bmult_rows(W, rows0, nrows):
                L = N - W
                v.tensor_tensor(
                    out=mk_ap(cb, BB(W) + 64 * rows0, [(CH, 2), (64, nrows), (1, L)]),
                    in0=mk_ap(cb, IRp + 64 * (rows0 + 1), [(-CH, 2), (64, nrows), (1, L)]),
                    in1=mk_ap(cb, FoR + 64 * (W - 1 - rows0) + rows0 + 1,
                              [(3 * CH, 2), (-63, nrows), (1, L)]),
                    op=MUL)

            def bpatch(W):
                L = N - W
                v.tensor_tensor(
                    out=mk_ap(cb, BB(W), [(CH, 2), (64 * (W - 2), 2), (1, L)]),
                    in0=mk_ap(cb, IRp + 64, [(-CH, 2), (64 * (W - 2), 2), (1, L)]),
                    in1=mk_ap(cb, FoR + 64 * (W - 1) + 1,
                              [(3 * CH, 2), (-63 * (W - 2), 2), (1, L)]),
                    op=MUL)

            def fold_b(w, c, npair):
                L = N - w
                v.tensor_tensor(
                    out=mk_ap(cb, BB(w), [(CH, 2), (64, npair), (1, L)]),
                    in0=mk_ap(cb, BB(w), [(CH, 2), (64, npair), (1, L)]),
                    in1=mk_ap(cb, BB(w) + 64 * c, [(CH, 2), (64, npair), (1, L)]),
                    op=ADD)

            def fold_a(w, c, npair):
                L = N - w
                v.tensor_tensor(
                    out=mk_ap(cb, SBA(w) + 64, [(64, npair), (1, L)]),
                    in0=mk_ap(cb, SBA(w) + 64, [(64, npair), (1, L)]),
                    in1=mk_ap(cb, SBA(w) + 64 + 64 * c, [(64, npair), (1, L)]),
                    op=ADD)

            def fseg(w, src):
                L = N - w
                v.tensor_tensor(
                    out=mk_ap(cb, FS, [(64, 2), (1, L)]),
                    in0=mk_ap(cb, src, [(64, 2), (1, L)]),
                    in1=mk_ap(cb, TFR + 64 * w, [(CH, 2), (1, L)]),
                    op=MUL)

            def irp(w, src):
                L = N - w
                v.tensor_tensor(
                    out=mk_ap(cb, IRp + 64 * w, [(CH, 2), (1, L)]),
                    in0=mk_ap(cb, src, [(64, 2), (1, L)]),
                    in1=mk_ap(cb, T3R + 64 * w, [(CH, 2), (1, L)]),
                    op=MUL)

            def quad_f(w):
                L = N - w
                v.scalar_tensor_tensor(
                    out=mk_ap(cb, FoR + 64 * w, [(CH, 2), (1, L)]),
                    in0=mk_ap(cb, BB(w), [(CH, 2), (1, L)]),
                    scalar=1.0,
                    in1=mk_ap(cb, FS, [(64, 2), (1, L)]),
                    op0=MUL, op1=ADD)

            def f_copy1():
                v.tensor_copy(
                    out=mk_ap(cb, FoR + 64, [(CH, 2), (1, 63)]),
                    in_=mk_ap(cb, FS, [(64, 2), (1, 63)]))

            def steady_step(w):
                la = fold_schedule(w - 2)   # shared A rows 1..w-2
                lb = fold_schedule(w - 1)   # B rows 0..w-2
                have_next = w + 1 < N
                Wn = w + 1
                if have_next:
                    nsh = w - 1            # shared rows 1..w-1 for Wn
                    m = max(nsh // 2, 1)
                # fold interleave: B1 A1 B2 A2 ... (trailing B allowed:
                # B_j+1 <- B_j at distance 2 via the A between them)
                for j in range(len(lb)):
                    fold_b(w, lb[j][0], lb[j][1])
                    if j < len(la):
                        fold_a(w, la[j][0], la[j][1])
                t1_add(w)
                if have_next:
                    sha_mult(Wn, 1, m)
                    sega_add(w)
                    if w - 2 >= 1:
                        bmult_rows(Wn, 1, w - 2)
                    else:
                        v.drain()
                    fseg(w, SGA)
                    irp(w, SGA)
                    quad_f(w)
                    if nsh - m >= 1:
                        sha_mult(Wn, 1 + m, nsh - m)
                    else:
                        v.drain()
                    bpatch(Wn)
                    apatch0(Wn)
                    apatchT(Wn)
                else:
                    v.drain()
                    sega_add(w)
                    v.drain()
                    fseg(w, SGA)
                    irp(w, SGA)
                    quad_f(w)

            def boot_w12_drains():
                apatch1()
                v.drain()
                fseg(1, P0(1))
                irp(1, P0(1))
                f_copy1()
                v.drain()
                bmult_rows(2, 0, 1)
                apatch0(2)
                apatchT(2)
                v.drain()
                t1_add(2)
                v.drain()
                fseg(2, T1)
                irp(2, T1)
                v.drain()
                quad_f(2)
                v.drain()
                bpatch(3)
                apatch0(3)
                apatchT(3)
                sha_mult(3, 1, 1)
                v.drain()

            def root_ops():
                ops = []
                ops.append(lambda: v.tensor_tensor(
                    out=mk_ap(cb, RTS, [(1, 2048)]),
                    in0=mk_ap(cb, FoR, [(1, 2048)]),
                    in1=mk_ap(cb, OH2, [(1, 2048)]),
                    op=MUL))
                ops.append(lambda: v.tensor_tensor(
                    out=mk_ap(cb, RTS + 2048, [(1, 2048)]),
                    in0=mk_ap(cb, FoR + 2048, [(1, 2048)]),
                    in1=mk_ap(cb, OH2 + 2048, [(1, 2048)]),
                    op=MUL))
                ops.append(lambda: v.tensor_copy(
                    out=mk_ap(cb, SNAP, [(1, 64)]),
                    in_=mk_ap(cb, FoL, [(64, 64)])))
                for c, npair in fold_schedule(32):
                    for half in (0, 2048):
                        ops.append(lambda half=half, c=c, npair=npair: v.tensor_tensor(
                            out=mk_ap(cb, RTS + half, [(64, npair), (1, 64)]),
                            in0=mk_ap(cb, RTS + half, [(64, npair), (1, 64)]),
                            in1=mk_ap(cb, RTS + half + 64 * c, [(64, npair), (1, 64)]),
                            op=ADD))
                ops.append(lambda: v.tensor_tensor(
                    out=mk_ap(cb, RTS, [(1, 64)]),
                    in0=mk_ap(cb, RTS, [(1, 64)]),
                    in1=mk_ap(cb, RTS + 2048, [(1, 64)]),
                    op=ADD))
                ops.append(lambda: v.tensor_tensor(
                    out=mk_ap(sf, RS1, [(1, 64)]),
                    in0=mk_ap(cb, RTS, [(1, 64)]),
                    in1=mk_ap(cb, SNAP, [(1, 64)]),
                    op=MUL))
                ops.append(lambda: v.tensor_tensor(
                    out=mk_ap(sf, RS2, [(1, 64)]),
                    in0=mk_ap(sf, RS1, [(1, 64)]),
                    in1=mk_ap(sf, ROOTT, [(1, 64)]),
                    op=MUL))
                ops.append(lambda: v.tensor_reduce(
                    out=pt[:], in_=mk_ap(sf, RS2, [(1, 64)]), axis=X, op=ADD))
                return ops

            def root_with_drains():
                for op in root_ops():
                    op()
                    v.drain()

            def boundary():
                b_ = [
                    lambda: apatch1(),              # b0
                    lambda: fseg(1, P0(1)),         # b1
                    lambda: irp(1, P0(1)),          # b2
                    lambda: f_copy1(),              # b3
                    lambda: bmult_rows(2, 0, 1),    # b4
                    lambda: apatch0(2),             # b5
                    lambda: apatchT(2),             # b6
                    lambda: t1_add(2),              # b7
                    lambda: fseg(2, T1),            # b8
                    lambda: irp(2, T1),             # b9
                    lambda: quad_f(2),              # b10
                    lambda: bpatch(3),              # b11
                    lambda: apatch0(3),             # b12
                    lambda: apatchT(3),             # b13
                    lambda: sha_mult(3, 1, 1),      # b14
                ]
                r_ = root_ops()
                order = [r_[0], r_[1], b_[0], r_[2], r_[3], b_[1], r_[4],
                         r_[5], b_[2], r_[6], r_[7], b_[3], r_[8], r_[9],
                         b_[4], r_[10], r_[11], b_[5], r_[12], b_[6],
                         r_[13], b_[7], r_[14], b_[8], r_[15], b_[9],
                         r_[16], b_[10], b_[14], b_[11], b_[12], b_[13]]
                for op in order:
                    op()

            # ---------------- program ----------------
            v.wait_ge(dsem, 32)
            boot_w12_drains()
            for rep in range(n_repeats):
                for w in range(3, N):
                    steady_step(w)
                if rep + 1 < n_repeats:
                    boundary()
                else:
                    v.drain()
                    root_with_drains()
            v.drain().then_inc(vsem, 1)

    nc.finalize()
    return nc


def prep_core_inputs(tag_array, len_array, root_param, trans_param, dec_param):
    th = np.asarray(tag_array)
    ln = np.asarray(len_array)
    tp = np.asarray(trans_param, np.float32)[..., 0]
    dec = np.asarray(dec_param, np.float32)
    root = np.asarray(root_param, np.float32)

    d = dec[th]
    goR_nc, goR_hc = d[:, :, RIGHT_, NC_, GO_], d[:, :, RIGHT_, HC_, GO_]
    goL_nc, goL_hc = d[:, :, LEFT_, NC_, GO_], d[:, :, LEFT_, HC_, GO_]
    stR_nc, stR_hc = d[:, :, RIGHT_, NC_, STOP_], d[:, :, RIGHT_, HC_, STOP_]
    stL_nc, stL_hc = d[:, :, LEFT_, NC_, STOP_], d[:, :, LEFT_, HC_, STOP_]
    trans_r = tp[th[:, :, None], th[:, None, :], RIGHT_]
    trans_l = tp[th[:, :, None], th[:, None, :], LEFT_]

    t3R = np.exp(trans_r + goR_hc[:, :, None] + stL_hc[:, None, :]
                 + stR_hc[:, None, :] + ALPHA, dtype=np.float32)
    t3L = np.exp(trans_l + goL_hc[:, :, None] + stR_hc[:, None, :]
                 + stL_hc[:, None, :] + ALPHA, dtype=np.float32)
    tfR = t3R * np.exp(stR_nc - stR_hc)[:, None, :]
    tfL = t3L * np.exp(stL_nc - stL_hc)[:, None, :]

    ar = np.arange(N)
    cbimg = np.zeros((B, CBF), np.float32)
    cbimg[:, CoR + ar] = np.exp(goR_nc - goR_hc)
    cbimg[:, CoL + ar] = np.exp(goL_nc - goL_hc)
    cbimg[:, FoR + ar] = np.exp(stR_nc - stR_hc)
    cbimg[:, FoL + ar] = np.exp(stL_nc - stL_hc)
    hh, mm = np.triu_indices(N, 1)
    off_r = 64 * (mm - hh) + hh
    cbimg[:, T3R + off_r] = t3R[:, hh, mm]
    cbimg[:, TFR + off_r] = tfR[:, hh, mm]
    lh, lm = np.tril_indices(N, -1)
    off_l = 64 * (lh - lm) + lm
    cbimg[:, T3L + off_l] = t3L[:, lh, lm]
    cbimg[:, TFL + off_l] = tfL[:, lh, lm]
    dd, ii = np.meshgrid(ar, ar, indexing="ij")
    mask = (dd + ii)[None, :, :] == (ln - 1)[:, None, None]
    cbimg[:, OH2:OH2 + CH] = mask.reshape(B, CH)
    cbimg = cbimg.astype(BF)

    sfimg = np.zeros((B, SFF), np.float32)
    sfimg[:, ROOTT + ar] = np.exp(root[th] + stL_hc + stR_hc) \
        * (ar[None, :] < ln[:, None])
    return ([cbimg[c * BPC:(c + 1) * BPC] for c in range(NCORES)],
            [sfimg[c * BPC:(c + 1) * BPC] for c in range(NCORES)])


_NC_CACHE = None


def kernel(id_array, tag_array, len_array, root_param, trans_param, dec_param):
    global _NC_CACHE
    if _NC_CACHE is None:
        # 2 repetitions: transient device flakes concentrate at program
        # start; the output ships from the self-healing second repetition.
        _NC_CACHE = build_nc(2)
    nc = _NC_CACHE
    cbs, sfs = prep_core_inputs(tag_array, len_array, root_param,
                                trans_param, dec_param)
    from concourse.bass_utils import run_bass_kernel_spmd
    in_maps = [{"inp": cbs[c], "inpf": sfs[c]} for c in range(NCORES)]
    P = None
    for attempt in range(3):
        res = run_bass_kernel_spmd(nc, in_maps, list(range(NCORES)))
        P = np.concatenate([np.asarray(res.results[c]["out"])[:, 0]
                            for c in range(NCORES)])
        if np.all(np.isfinite(P)) and np.all(P > 0):
            break
    ln = np.asarray(len_array)
    ll = np.log(P) - ALPHA * (ln - 1)
    return ll.astype(np.float32)


# revision 4
# speedup vs baseline: 1.1335x; 1.0082x over previous
"""Trainium2 Bass kernel (v8) for batched DMV inside.

v7 (drain-free pipeline, C-chart elimination) plus shared A-bands:
since C(d) == F(d) for d >= 1, the A-band interior rows are identical
between directions: bandA_R[k,i] = FR(k,i)*FL(w-1-k,i+k+1) =
bandA_L[k,i] for k = 1..w-2. Those rows are computed and folded ONCE
(single-block band SBA); only row 0 (C-diag0 special) and row w-1
(F'-diag0 special) are direction-specific, kept as 2x64 scratch rows
P0/PT. segA[dir] = (P0[dir]+PT[dir]) + M where M is the shared fold
result (broadcast across dirs with a stride-0 AP dim).
"""
import numpy as np
import ml_dtypes
import bass_rust
import concourse.bass as bass
import concourse.mybir as mybir

F32 = mybir.dt.float32
BF16 = mybir.dt.bfloat16
BF = ml_dtypes.bfloat16
MUL = mybir.AluOpType.mult
ADD = mybir.AluOpType.add
X = mybir.AxisListType.X

N = 64
B = 1024
NCORES = 8
BPC = B // NCORES
ALPHA = 5.0
NC_, HC_, GO_, STOP_, LEFT_, RIGHT_ = 0, 1, 0, 1, 0, 1

CH = 4096
CoR, CoL, FoR, FoL = 0, CH, 2 * CH, 3 * CH
IRp, ILp = 4 * CH, 5 * CH
T3R, T3L = 6 * CH, 7 * CH
TFR, TFL = 8 * CH, 9 * CH
OH2 = 10 * CH
SBA0 = 11 * CH           # shared A band, parity 0 (single block, 64 rows)
BB0 = 13 * CH            # B band parity 0 (R at BB0, L at BB0+CH)
SBA1 = 15 * CH
BB1 = 17 * CH
RTS = 19 * CH            # root-phase scratch (2 x 2048 halves)
SCR = 20 * CH            # small scratch block
FS = SCR                 # fseg [2dir x 64]
P0_0 = SCR + 128         # A row 0 (dir-specific), parity 0
PT_0 = SCR + 256         # A row W-1, parity 0
P0_1 = SCR + 384
PT_1 = SCR + 512
T1 = SCR + 640           # P0+PT
SGA = SCR + 768          # segA [2dir x 64]
SNAP = SCR + 896         # FoL[d,0] snapshot for the boundary root
CBF = SCR + 960

ROOTT, RS1, RS2 = 0, 64, 128
SFF = 192


def mk_ap(t, offset, dims):
    a = t[:]
    fsz = a.ap[0][0]
    a.ap = bass_rust.VecI64Pair([[fsz, 128]] + [list(d) for d in dims])
    a.offset = offset
    return a


def fold_schedule(rows):
    ops, r = [], rows
    while r > 1:
        c = (r + 1) // 2
        ops.append((c, r - c))
        r = c
    return ops


def SBA(w):
    return SBA0 if (w & 1) == 0 else SBA1


def BB(w):
    return BB0 if (w & 1) == 0 else BB1


def P0(w):
    return P0_0 if (w & 1) == 0 else P0_1


def PT(w):
    return PT_0 if (w & 1) == 0 else PT_1


def build_nc(n_repeats: int = 1):
    nc = bass.Bass()
    inp = nc.dram_tensor("inp", [BPC, CBF], BF16, kind="ExternalInput")
    inpf = nc.dram_tensor("inpf", [BPC, SFF], F32, kind="ExternalInput")
    outp = nc.dram_tensor("out", [BPC, 1], F32, kind="ExternalOutput")

    cb = nc.alloc_sbuf_tensor("cb", [128, CBF], BF16)
    sf = nc.alloc_sbuf_tensor("sf", [128, SFF], F32)
    pt = nc.alloc_sbuf_tensor("pt", [128, 1], F32)

    with (
        nc.Block() as block,
        nc.semaphore("dsem") as dsem,
        nc.semaphore("vsem") as vsem,
    ):
        @block.sync
        def _(sync):
            sync.dma_start(out=cb[:], in_=inp[:]).then_inc(dsem, 16)
            sync.dma_start(out=sf[:], in_=inpf[:]).then_inc(dsem, 16)
            sync.wait_ge(vsem, 1)
            sync.dma_start(out=outp[:], in_=pt[:]).then_inc(dsem, 16)

        @block.vector
        def _(v):
            def sha_mult(W, rows0, nrows):
                """Shared A rows rows0..rows0+nrows-1 (rows0 >= 1):
                row k = F(k,i) * F'(W-1-k, i+k+1), same for both dirs."""
                L = N - W
                v.tensor_tensor(
                    out=mk_ap(cb, SBA(W) + 64 * rows0, [(64, nrows), (1, L)]),
                    in0=mk_ap(cb, FoR + 64 * rows0, [(64, nrows), (1, L)]),
                    in1=mk_ap(cb, FoL + 64 * (W - 1 - rows0) + rows0 + 1,
                              [(-63, nrows), (1, L)]),
                    op=MUL)

            def apatch1():
                """w=1 band row (row 0 == row W-1): both operands are
                diag-0 specials -> P0(1)."""
                v.tensor_tensor(
                    out=mk_ap(cb, P0(1), [(64, 2), (1, 63)]),
                    in0=mk_ap(cb, CoR, [(2 * CH, 2), (1, 63)]),
                    in1=mk_ap(cb, FoL + 1, [(-2 * CH, 2), (1, 63)]),
                    op=MUL)

            def apatch0(W):
                """A row 0 -> P0(W): C(0)/F(0) specials x F'(W-1)."""
                L = N - W
                v.tensor_tensor(
                    out=mk_ap(cb, P0(W), [(64, 2), (1, L)]),
                    in0=mk_ap(cb, CoR, [(2 * CH, 2), (1, L)]),
                    in1=mk_ap(cb, FoL + 64 * (W - 1) + 1, [(0, 2), (1, L)]),
                    op=MUL)

            def apatchT(W):
                """A row W-1 -> PT(W): F(W-1) x C'(0)/F'(0) specials."""
                L = N - W
                v.tensor_tensor(
                    out=mk_ap(cb, PT(W), [(64, 2), (1, L)]),
                    in0=mk_ap(cb, FoR + 64 * (W - 1), [(0, 2), (1, L)]),
                    in1=mk_ap(cb, FoL + W, [(-2 * CH, 2), (1, L)]),
                    op=MUL)

            def t1_add(w):
                L = N - w
                v.tensor_tensor(
                    out=mk_ap(cb, T1, [(64, 2), (1, L)]),
                    in0=mk_ap(cb, P0(w), [(64, 2), (1, L)]),
                    in1=mk_ap(cb, PT(w), [(64, 2), (1, L)]),
                    op=ADD)

            def sega_add(w):
                L = N - w
                v.tensor_tensor(
                    out=mk_ap(cb, SGA, [(64, 2), (1, L)]),
                    in0=mk_ap(cb, T1, [(64, 2), (1, L)]),
                    in1=mk_ap(cb, SBA(w) + 64, [(0, 2), (1, L)]),
                    op=ADD)

            def bmult_rows(W, rows0, nrows):
                L = N - W
                v.tensor_tensor(
                    out=mk_ap(cb, BB(W) + 64 * rows0, [(CH, 2), (64, nrows), (1, L)]),
                    in0=mk_ap(cb, IRp + 64 * (rows0 + 1), [(-CH, 2), (64, nrows), (1, L)]),
                    in1=mk_ap(cb, FoR + 64 * (W - 1 - rows0) + rows0 + 1,
                              [(3 * CH, 2), (-63, nrows), (1, L)]),
                    op=MUL)

            def bpatch(W):
                L = N - W
                v.tensor_tensor(
                    out=mk_ap(cb, BB(W), [(CH, 2), (64 * (W - 2), 2), (1, L)]),
                    in0=mk_ap(cb, IRp + 64, [(-CH, 2), (64 * (W - 2), 2), (1, L)]),
                    in1=mk_ap(cb, FoR + 64 * (W - 1) + 1,
                              [(3 * CH, 2), (-63 * (W - 2), 2), (1, L)]),
                    op=MUL)

            def fold_b(w, c, npair):
                L = N - w
                v.tensor_tensor(
                    out=mk_ap(cb, BB(w), [(CH, 2), (64, npair), (1, L)]),
                    in0=mk_ap(cb, BB(w), [(CH, 2), (64, npair), (1, L)]),
                    in1=mk_ap(cb, BB(w) + 64 * c, [(CH, 2), (64, npair), (1, L)]),
                    op=ADD)

            def fold_a(w, c, npair):
                L = N - w
                v.tensor_tensor(
                    out=mk_ap(cb, SBA(w) + 64, [(64, npair), (1, L)]),
                    in0=mk_ap(cb, SBA(w) + 64, [(64, npair), (1, L)]),
                    in1=mk_ap(cb, SBA(w) + 64 + 64 * c, [(64, npair), (1, L)]),
                    op=ADD)

            def fseg(w, src):
                L = N - w
                v.tensor_tensor(
                    out=mk_ap(cb, FS, [(64, 2), (1, L)]),
                    in0=mk_ap(cb, src, [(64, 2), (1, L)]),
                    in1=mk_ap(cb, TFR + 64 * w, [(CH, 2), (1, L)]),
                    op=MUL)

            def irp(w, src):
                L = N - w
                v.tensor_tensor(
                    out=mk_ap(cb, IRp + 64 * w, [(CH, 2), (1, L)]),
                    in0=mk_ap(cb, src, [(64, 2), (1, L)]),
                    in1=mk_ap(cb, T3R + 64 * w, [(CH, 2), (1, L)]),
                    op=MUL)

            def quad_f(w):
                L = N - w
                v.scalar_tensor_tensor(
                    out=mk_ap(cb, FoR + 64 * w, [(CH, 2), (1, L)]),
                    in0=mk_ap(cb, BB(w), [(CH, 2), (1, L)]),
                    scalar=1.0,
                    in1=mk_ap(cb, FS, [(64, 2), (1, L)]),
                    op0=MUL, op1=ADD)

            def f_copy1():
                v.tensor_copy(
                    out=mk_ap(cb, FoR + 64, [(CH, 2), (1, 63)]),
                    in_=mk_ap(cb, FS, [(64, 2), (1, 63)]))

            def steady_step(w):
                la = fold_schedule(w - 2)   # shared A rows 1..w-2
                lb = fold_schedule(w - 1)   # B rows 0..w-2
                have_next = w + 1 < N
                Wn = w + 1
                if have_next:
                    nsh = w - 1            # shared rows 1..w-1 for Wn
                    m = max(nsh // 2, 1)
                # fold interleave: B1 A1 B2 A2 ... (trailing B allowed:
                # B_j+1 <- B_j at distance 2 via the A between them)
                for j in range(len(lb)):
                    fold_b(w, lb[j][0], lb[j][1])
                    if j < len(la):
                        fold_a(w, la[j][0], la[j][1])
                t1_add(w)
                if have_next:
                    sha_mult(Wn, 1, m)
                    sega_add(w)
                    if w - 2 >= 1:
                        bmult_rows(Wn, 1, w - 2)
                    else:
                        v.drain()
                    fseg(w, SGA)
                    irp(w, SGA)
                    quad_f(w)
                    if nsh - m >= 1:
                        sha_mult(Wn, 1 + m, nsh - m)
                    else:
                        v.drain()
                    bpatch(Wn)
                    apatch0(Wn)
                    apatchT(Wn)
                else:
                    v.drain()
                    sega_add(w)
                    v.drain()
                    fseg(w, SGA)
                    irp(w, SGA)
                    quad_f(w)

            def boot_w12_drains():
                apatch1()
                v.drain()
                fseg(1, P0(1))
                irp(1, P0(1))
                f_copy1()
                v.drain()
                bmult_rows(2, 0, 1)
                apatch0(2)
                apatchT(2)
                v.drain()
                t1_add(2)
                v.drain()
                fseg(2, T1)
                irp(2, T1)
                v.drain()
                quad_f(2)
                v.drain()
                bpatch(3)
                apatch0(3)
                apatchT(3)
                sha_mult(3, 1, 1)
                v.drain()

            def root_ops():
                ops = []
                ops.append(lambda: v.tensor_tensor(
                    out=mk_ap(cb, RTS, [(1, 2048)]),
                    in0=mk_ap(cb, FoR, [(1, 2048)]),
                    in1=mk_ap(cb, OH2, [(1, 2048)]),
                    op=MUL))
                ops.append(lambda: v.tensor_tensor(
                    out=mk_ap(cb, RTS + 2048, [(1, 2048)]),
                    in0=mk_ap(cb, FoR + 2048, [(1, 2048)]),
                    in1=mk_ap(cb, OH2 + 2048, [(1, 2048)]),
                    op=MUL))
                ops.append(lambda: v.tensor_copy(
                    out=mk_ap(cb, SNAP, [(1, 64)]),
                    in_=mk_ap(cb, FoL, [(64, 64)])))
                for c, npair in fold_schedule(32):
                    for half in (0, 2048):
                        ops.append(lambda half=half, c=c, npair=npair: v.tensor_tensor(
                            out=mk_ap(cb, RTS + half, [(64, npair), (1, 64)]),
                            in0=mk_ap(cb, RTS + half, [(64, npair), (1, 64)]),
                            in1=mk_ap(cb, RTS + half + 64 * c, [(64, npair), (1, 64)]),
                            op=ADD))
                ops.append(lambda: v.tensor_tensor(
                    out=mk_ap(cb, RTS, [(1, 64)]),
                    in0=mk_ap(cb, RTS, [(1, 64)]),
                    in1=mk_ap(cb, RTS + 2048, [(1, 64)]),
                    op=ADD))
                ops.append(lambda: v.tensor_tensor(
                    out=mk_ap(sf, RS1, [(1, 64)]),
                    in0=mk_ap(cb, RTS, [(1, 64)]),
                    in1=mk_ap(cb, SNAP, [(1, 64)]),
                    op=MUL))
                ops.append(lambda: v.tensor_tensor(
                    out=mk_ap(sf, RS2, [(1, 64)]),
                    in0=mk_ap(sf, RS1, [(1, 64)]),
                    in1=mk_ap(sf, ROOTT, [(1, 64)]),
                    op=MUL))
                ops.append(lambda: v.tensor_reduce(
                    out=pt[:], in_=mk_ap(sf, RS2, [(1, 64)]), axis=X, op=ADD))
                return ops

            def root_with_drains():
                for op in root_ops():
                    op()
                    v.drain()

            def boundary():
                b_ = [
                    lambda: apatch1(),              # b0
                    lambda: fseg(1, P0(1)),         # b1
                    lambda: irp(1, P0(1)),          # b2
                    lambda: f_copy1(),              # b3
                    lambda: bmult_rows(2, 0, 1),    # b4
                    lambda: apatch0(2),             # b5
                    lambda: apatchT(2),             # b6
                    lambda: t1_add(2),              # b7
                    lambda: fseg(2, T1),            # b8
                    lambda: irp(2, T1),             # b9
                    lambda: quad_f(2),              # b10
                    lambda: bpatch(3),              # b11
                    lambda: apatch0(3),             # b12
                    lambda: apatchT(3),             # b13
                    lambda: sha_mult(3, 1, 1),      # b14
                ]
                r_ = root_ops()
                order = [r_[0], r_[1], b_[0], r_[2], r_[3], b_[1], r_[4],
                         r_[5], b_[2], r_[6], r_[7], b_[3], r_[8], r_[9],
                         b_[4], r_[10], r_[11], b_[5], r_[12], b_[6],
                         r_[13], b_[7], r_[14], b_[8], r_[15], b_[9],
                         r_[16], b_[10], b_[14], b_[11], b_[12], b_[13]]
                for op in order:
                    op()

            # ---------------- program ----------------
            v.wait_ge(dsem, 32)
            boot_w12_drains()
            for rep in range(n_repeats):
                for w in range(3, N):
                    steady_step(w)
                if rep + 1 < n_repeats:
                    boundary()
                else:
                    v.drain()
                    root_with_drains()
            v.drain().then_inc(vsem, 1)

    nc.finalize()
    return nc


def prep_core_inputs(tag_array, len_array, root_param, trans_param, dec_param):
    th = np.asarray(tag_array)
    ln = np.asarray(len_array)
    tp = np.asarray(trans_param, np.float32)[..., 0]
    dec = np.asarray(dec_param, np.float32)
    root = np.asarray(root_param, np.float32)

    d = dec[th]
    goR_nc, goR_hc = d[:, :, RIGHT_, NC_, GO_], d[:, :, RIGHT_, HC_, GO_]
    goL_nc, goL_hc = d[:, :, LEFT_, NC_, GO_], d[:, :, LEFT_, HC_, GO_]
    stR_nc, stR_hc = d[:, :, RIGHT_, NC_, STOP_], d[:, :, RIGHT_, HC_, STOP_]
    stL_nc, stL_hc = d[:, :, LEFT_, NC_, STOP_], d[:, :, LEFT_, HC_, STOP_]
    trans_r = tp[th[:, :, None], th[:, None, :], RIGHT_]
    trans_l = tp[th[:, :, None], th[:, None, :], LEFT_]

    t3R = np.exp(trans_r + goR_hc[:, :, None] + stL_hc[:, None, :]
                 + stR_hc[:, None, :] + ALPHA, dtype=np.float32)
    t3L = np.exp(trans_l + goL_hc[:, :, None] + stR_hc[:, None, :]
                 + stL_hc[:, None, :] + ALPHA, dtype=np.float32)
    tfR = t3R * np.exp(stR_nc - stR_hc)[:, None, :]
    tfL = t3L * np.exp(stL_nc - stL_hc)[:, None, :]

    ar = np.arange(N)
    cbimg = np.zeros((B, CBF), np.float32)
    cbimg[:, CoR + ar] = np.exp(goR_nc - goR_hc)
    cbimg[:, CoL + ar] = np.exp(goL_nc - goL_hc)
    cbimg[:, FoR + ar] = np.exp(stR_nc - stR_hc)
    cbimg[:, FoL + ar] = np.exp(stL_nc - stL_hc)
    hh, mm = np.triu_indices(N, 1)
    off_r = 64 * (mm - hh) + hh
    cbimg[:, T3R + off_r] = t3R[:, hh, mm]
    cbimg[:, TFR + off_r] = tfR[:, hh, mm]
    lh, lm = np.tril_indices(N, -1)
    off_l = 64 * (lh - lm) + lm
    cbimg[:, T3L + off_l] = t3L[:, lh, lm]
    cbimg[:, TFL + off_l] = tfL[:, lh, lm]
    dd, ii = np.meshgrid(ar, ar, indexing="ij")
    mask = (dd + ii)[None, :, :] == (ln - 1)[:, None, None]
    cbimg[:, OH2:OH2 + CH] = mask.reshape(B, CH)
    cbimg = cbimg.astype(BF)

    sfimg = np.zeros((B, SFF), np.float32)
    sfimg[:, ROOTT + ar] = np.exp(root[th] + stL_hc + stR_hc) \
        * (ar[None, :] < ln[:, None])
    return ([cbimg[c * BPC:(c + 1) * BPC] for c in range(NCORES)],
            [sfimg[c * BPC:(c + 1) * BPC] for c in range(NCORES)])


_NC_CACHE = None


def kernel(id_array, tag_array, len_array, root_param, trans_param, dec_param):
    global _NC_CACHE
    if _NC_CACHE is None:
        # 3 repetitions: transient device flakes concentrate at program
        # start; the output ships from the self-healing final repetition.
        _NC_CACHE = build_nc(3)
    nc = _NC_CACHE
    cbs, sfs = prep_core_inputs(tag_array, len_array, root_param,
                                trans_param, dec_param)
    from concourse.bass_utils import run_bass_kernel_spmd
    in_maps = [{"inp": cbs[c], "inpf": sfs[c]} for c in range(NCORES)]
    P = None
    for attempt in range(3):
        res = run_bass_kernel_spmd(nc, in_maps, list(range(NCORES)))
        P = np.concatenate([np.asarray(res.results[c]["out"])[:, 0]
                            for c in range(NCORES)])
        if np.all(np.isfinite(P)) and np.all(P > 0):
            break
    ln = np.asarray(len_array)
    ll = np.log(P) - ALPHA * (ln - 1)
    return ll.astype(np.float32)
